# revision 10
# baseline (speedup 1.0000x reference)
"""DynamicConvolution Trainium2 kernel (8 NeuronCores, data-parallel over batch).

Reference computation (per sample b):
  pooled = mean(x[b], spatial); h = relu(pooled @ w1 + b1)
  alpha  = softmax(h @ w2 + b2)                   [8]
  W[b]   = sum_k alpha[k] * kernels[k]            [256,256,3,3]
  y[b]   = conv2d(x[b], W[b], pad=1)              [256,56,56]

Sharding: batch 16 -> 2 samples per core; bank + MLP weights replicated.

Device mapping (per core, bf16 compute, fp32 PSUM accumulation):
  - x arrives host-padded as [2,256,58*58] bf16 -> contiguous DMAs.
    DMA issue order x(b0), bank(o_t=0), x(b1), bank(o_t=1) minimizes the
    critical path: sample-0 attention runs while bank(o0) streams, so
    mixing matmuls start ~15us in.
  - attention is duplicated per sample (tiny) so alpha[b0] never waits on
    x[b1]: pooled via chunked VectorE reduces; MLP on TensorE with channels
    on partitions; biases as K=1 rank-1 matmuls; softmax VectorE/ScalarE;
    alphaT_b[128,1] (alphaT[c*8+k]=alpha[b,k]) via a K=8 selection matmul.
  - kernel mixing ON TensorE: per (b, i_tile, o_tile, tap-group) PSUM block
    [128i, g*128] accumulates 8 matmuls; matmul j uses a masked lhsT
    (alpha at ((c,k), 16j+c)) against the (c,k)-partition bank layout, so
    16 i-rows mix per streamed column.  VectorE evacuates straight into
    conv weight layout [i128, (s,o)] bf16.
  - conv: per (o_t, b, t) PSUM group of 18 accumulating matmuls
    [o128,448] += W[i128,o128]^T @ xpad[i128, 8x56 window]; ScalarE
    evacuates fp32; DMA out.

Sync discipline (walrus permits ONE semaphore wait per engine instruction):
  - matmul waits split: lhsT dep on InstLdweights, rhs dep on InstMatmult;
    PSUM-reuse WAR rides InstMatmult, so every matmul's rhs DMA queue is
    pre-observed by a 1x1 "touch" matmul, ordered with add_dep_helper.
  - mix-block PSUM evac on VectorE (conv LW waits merge on its sem);
    conv PSUM evac on ScalarE (conv MM WAR waits on its sem).
  - all tiny MLP/selection PSUM tiles share one bank tag sequentially; the
    touch scratch lives in its own bank with PE-local WAW only.
"""

import numpy as np
import ml_dtypes
from contextlib import ExitStack

try:
    import concourse.bass as bass
except ImportError:  # fresh grading dir: repo paths not on sys.path yet
    import sys
    for p in ("/opt/trn_rl_repo", "/root/.axon_site/_ro/trn_rl_repo"):
        if p not in sys.path:
            sys.path.append(p)
    import concourse.bass as bass

import concourse.mybir as mybir
import concourse.tile as tile
from concourse import bacc
from concourse.tile import add_dep_helper
from concourse.bass_utils import run_bass_kernel_spmd

F32 = mybir.dt.float32
BF16 = mybir.dt.bfloat16
AX = mybir.AxisListType.X
RELU = mybir.ActivationFunctionType.Relu
EXP = mybir.ActivationFunctionType.Exp
COPY = mybir.ActivationFunctionType.Copy

N_CORES = 8
B = 2               # samples per core
C = 256             # channels
IT = 2              # 128-channel input tiles
OT = 2              # 128-channel output tiles
H = W_IMG = 56
HW = H * W_IMG      # 3136
PADW = 58
PADHW = PADW * PADW  # 3364
NT = 7              # row blocks per image
TB = 448            # 8 rows x 56 cols per conv psum block
S = 9               # conv taps
WSL = S * 128       # 1152 = per (b,o_t,i_t) weight-slice elems
XCH = 4             # x DMA chunks per (b, i_tile)
XC = PADHW // XCH   # 841 elems per x chunk
SGRP = [(0, 4), (4, 4), (8, 1)]   # tap groups (start, len)

# mlp consts layout (fp32 [128, 336])
C_W1A, C_W1B, C_W2, C_B1, C_B2, C_ONES = 0, 64, 128, 136, 200, 208
C_COLS = 336
# mix consts layout (bf16 [128, 1152]): 8 masks | KSEL
M_MASK, M_KSEL = 0, 1024
M_COLS = 1152

_cached = None


def _build():
    nc = bacc.Bacc()
    xin = nc.declare_dram_parameter("x", [B, C, PADHW], BF16, isOutput=False)
    bankin = nc.declare_dram_parameter("bank", [128, OT * IT * 8 * WSL], BF16,
                                       isOutput=False)
    cin = nc.declare_dram_parameter("consts", [128, C_COLS], F32, isOutput=False)
    min_ = nc.declare_dram_parameter("mconsts", [128, M_COLS], BF16,
                                     isOutput=False)
    y = nc.declare_dram_parameter("y", [B, C, HW], F32, isOutput=True)

    with tile.TileContext(nc) as tc, ExitStack() as ctx:
        sb = ctx.enter_context(tc.tile_pool(name="sb", bufs=1))
        conv_ps = ctx.enter_context(tc.tile_pool(name="cps", bufs=4, space="PSUM"))
        mix_ps = ctx.enter_context(tc.tile_pool(name="xps", bufs=2, space="PSUM"))
        mlp_ps = ctx.enter_context(tc.tile_pool(name="mps", bufs=1, space="PSUM"))
        scr_ps = ctx.enter_context(tc.tile_pool(name="sps", bufs=1, space="PSUM"))

        xpad = sb.tile([128, B * IT * PADHW], BF16, tag="xpad")
        bank = sb.tile([128, OT * IT * 8 * WSL], BF16, tag="bank")
        wsb = sb.tile([128, B * OT * IT * WSL], BF16, tag="wsb")
        outsb = sb.tile([128, OT * B * NT * TB], F32, tag="outsb")
        consts = sb.tile([128, C_COLS], F32, tag="consts")
        mconsts = sb.tile([128, M_COLS], BF16, tag="mconsts")
        scratch = scr_ps.tile([1, 1], F32)

        def xv(b, it):
            base = (b * IT + it) * PADHW
            return xpad[:, base:base + PADHW].rearrange("p (r c) -> p r c", c=PADW)

        def pe_touch(ap):
            return nc.tensor.matmul(scratch[:], ap, ap, start=True, stop=True,
                                    skip_group_check=True)

        # ---------- DMAs: consts, then x(b0) | bank(o0) | x(b1) | bank(o1)
        nc.sync.dma_start(consts[:], cin[:])
        nc.sync.dma_start(mconsts[:], min_[:])

        def dma_x(b):
            for it in range(IT):
                base = (b * IT + it) * PADHW
                for cch in range(XCH):
                    nc.sync.dma_start(
                        xpad[:, base + cch * XC: base + (cch + 1) * XC],
                        xin[b, it * 128:(it + 1) * 128,
                            cch * XC:(cch + 1) * XC])

        def dma_bank(ot):
            for it in range(IT):
                for jp in range(4):
                    off = (((ot * IT + it) * 8) + 2 * jp) * WSL
                    nc.sync.dma_start(bank[:, off:off + 2 * WSL],
                                      bankin[:, off:off + 2 * WSL])

        dma_x(0)
        dma_bank(0)
        dma_x(1)
        dma_bank(1)

        # ---------- PE touches for x chunks (pre-observe DMA queues)
        xtouch = []
        for b in range(B):
            for it in range(IT):
                base = (b * IT + it) * PADHW
                for cch in range(XCH):
                    xtouch.append(pe_touch(
                        xpad[0:1, base + cch * XC: base + cch * XC + 1]))
        for t1, t0 in zip(xtouch[1:], xtouch[:-1]):
            add_dep_helper(t1.ins, t0.ins, sync=False, reason="touch chain")

        # ---------- per-sample attention -> alphaT -> masked lhsT tiles
        mtch = sb.tile([1, 1], BF16, tag="mtch")
        mtch_i = nc.vector.tensor_copy(mtch[:], mconsts[0:1, 0:1])
        partials = sb.tile([128, B * IT * XCH], F32, tag="partials")
        psum2 = sb.tile([128, B * IT], F32, tag="psum2")
        pooled = sb.tile([128, B * IT], F32, tag="pooled")   # col (b, it)
        lhsT = sb.tile([128, 16 * 128], BF16, tag="lhsT")    # (j, b) tiles

        for b in range(B):
            for it in range(IT):
                base = (b * IT + it) * PADHW
                for cch in range(XCH):
                    j = (b * IT + it) * XCH + cch
                    nc.vector.reduce_sum(
                        partials[:, j:j + 1],
                        xpad[:, base + cch * XC: base + (cch + 1) * XC], axis=AX)
            for it in range(IT):
                j = b * IT + it
                nc.vector.reduce_sum(psum2[:, j:j + 1],
                                     partials[:, j * XCH:(j + 1) * XCH], axis=AX)
                nc.vector.tensor_scalar_mul(pooled[:, j:j + 1],
                                            psum2[:, j:j + 1], 1.0 / HW)

            hT_ps = mlp_ps.tile([64, 1], F32, tag="mlp")
            nc.tensor.matmul(hT_ps[:], consts[0:1, C_B1:C_B1 + 64],
                             consts[0:1, C_ONES:C_ONES + 1],
                             start=True, stop=False)
            nc.tensor.matmul(hT_ps[:], consts[:, C_W1A:C_W1A + 64],
                             pooled[:, b * IT: b * IT + 1],
                             start=False, stop=False)
            nc.tensor.matmul(hT_ps[:], consts[:, C_W1B:C_W1B + 64],
                             pooled[:, b * IT + 1: b * IT + 2],
                             start=False, stop=True)
            hT = sb.tile([64, 1], F32, tag=f"hTs{b}")
            nc.scalar.activation(hT[:], hT_ps[:], RELU)

            sc_ps = mlp_ps.tile([1, 8], F32, tag="mlp")
            nc.tensor.matmul(sc_ps[:], consts[0:1, C_ONES:C_ONES + 1],
                             consts[0:1, C_B2:C_B2 + 8], start=True, stop=False)
            nc.tensor.matmul(sc_ps[:], hT[:], consts[0:64, C_W2:C_W2 + 8],
                             start=False, stop=True)

            scores = sb.tile([1, 8], F32, tag=f"scores{b}")
            nc.vector.tensor_copy(scores[:], sc_ps[:])
            mx = sb.tile([1, 1], F32, tag=f"mx{b}")
            nc.vector.reduce_max(mx[:], scores[:], axis=AX)
            subb = sb.tile([1, 8], F32, tag=f"subb{b}")
            nc.vector.tensor_scalar_sub(subb[:], scores[:], mx[:])
            ex = sb.tile([1, 8], F32, tag=f"ex{b}")
            nc.scalar.activation(ex[:], subb[:], EXP)
            z = sb.tile([1, 1], F32, tag=f"z{b}")
            nc.vector.reduce_sum(z[:], ex[:], axis=AX)
            rz = sb.tile([1, 1], F32, tag=f"rz{b}")
            nc.vector.reciprocal(rz[:], z[:])
            al = sb.tile([1, 8], F32, tag=f"al{b}")
            nc.vector.tensor_scalar_mul(al[:], ex[:], rz[:])

            a8 = sb.tile([8, 1], F32, tag=f"a8{b}")
            nc.sync.dma_start(a8[:], al[:])          # [1,8] -> [8,1]
            a8h = sb.tile([8, 1], BF16, tag=f"a8h{b}")
            a8h_i = nc.vector.tensor_copy(a8h[:], a8[:])
            if b == 0:
                add_dep_helper(a8h_i.ins, mtch_i.ins, sync=False,
                               reason="mconsts seen on DVE")
            aT_ps = mlp_ps.tile([128, 1], F32, tag="mlp")
            nc.tensor.matmul(aT_ps[:], mconsts[0:8, M_KSEL:M_KSEL + 128],
                             a8h[:], start=True, stop=True)
            aT = sb.tile([128, 1], F32, tag=f"aT{b}")
            nc.vector.tensor_copy(aT[:], aT_ps[:])
            for j in range(8):
                nc.vector.tensor_scalar_mul(
                    lhsT[:, (j * B + b) * 128:(j * B + b + 1) * 128],
                    mconsts[:, M_MASK + j * 128:M_MASK + (j + 1) * 128],
                    aT[:, 0:1])

        # ---------- per o_tile: bank touches, PE mixing, conv
        for ot in range(OT):
            btch = []
            for it in range(IT):
                for jp in range(4):
                    off = (((ot * IT + it) * 8) + 2 * jp) * WSL
                    btch.append(pe_touch(bank[0:1, off:off + 1]))
            for t1, t0 in zip(btch[1:], btch[:-1]):
                add_dep_helper(t1.ins, t0.ins, sync=False, reason="btouch chain")
            if ot == 0:
                add_dep_helper(btch[0].ins, xtouch[-1].ins, sync=False,
                               reason="after x touches")

            # mixing: blocks (b, it, grp); 8 accumulated matmuls each
            for b in range(B):
                for it in range(IT):
                    woff = ((b * OT + ot) * IT + it) * WSL
                    for (s0, g) in SGRP:
                        mps = mix_ps.tile([128, 512], F32, tag="mix")
                        n = g * 128
                        for j in range(8):
                            boff = ((ot * IT + it) * 8 + j) * WSL + s0 * 128
                            mm = nc.tensor.matmul(
                                mps[:, 0:n],
                                lhsT[:, (j * B + b) * 128:(j * B + b + 1) * 128],
                                bank[:, boff:boff + n],
                                start=(j == 0), stop=(j == 7))
                            if j == 0:
                                add_dep_helper(mm.ins, btch[-1].ins, sync=False,
                                               reason="bank observed")
                        nc.vector.tensor_copy(
                            wsb[:, woff + s0 * 128: woff + (s0 + g) * 128],
                            mps[:, 0:n])

            # conv groups (b, t): 18 accumulated matmuls each
            for b in range(B):
                for t in range(NT):
                    ps = conv_ps.tile([128, TB], F32, tag="convps")
                    n_mm = 0
                    for it in range(IT):
                        woff = ((b * OT + ot) * IT + it) * WSL
                        v = xv(b, it)
                        for s in range(S):
                            kh, kw = s // 3, s % 3
                            nc.tensor.matmul(
                                ps[:],
                                wsb[:, woff + s * 128: woff + (s + 1) * 128],
                                v[:, 8 * t + kh: 8 * t + kh + 8, kw:kw + 56],
                                start=(n_mm == 0), stop=(n_mm == 17))
                            n_mm += 1
                    blk = (ot * B + b) * NT + t
                    nc.scalar.activation(outsb[:, blk * TB:(blk + 1) * TB],
                                         ps[:], COPY)
                    nc.sync.dma_start(
                        y[b, ot * 128:(ot + 1) * 128, t * TB:(t + 1) * TB],
                        outsb[:, blk * TB:(blk + 1) * TB])

    nc.compile()
    return nc


def _prep(x, kernels, w1, b1, w2, b2):
    """Host-side marshaling: dtype casts + layout/padding rearrangement only."""
    xp = np.zeros((16, C, PADW, PADW), dtype=ml_dtypes.bfloat16)
    xp[:, :, 1:57, 1:57] = x
    xh = np.ascontiguousarray(xp.reshape(16, C, PADHW))
    # bank[(c,k), (ot, it, j, s, o)] = kernels[k, ot*128+o, it*128+16j+c, s]
    kr = kernels.reshape(8, OT, 128, IT, 8, 16, S)   # k,ot,o,it,j,c,s
    bankh = np.ascontiguousarray(
        kr.transpose(5, 0, 1, 3, 4, 6, 2).reshape(128, OT * IT * 8 * WSL)
        .astype(ml_dtypes.bfloat16))
    consts = np.zeros((128, C_COLS), dtype=np.float32)
    consts[:, C_W1A:C_W1A + 64] = w1[0:128]
    consts[:, C_W1B:C_W1B + 64] = w1[128:256]
    consts[0:64, C_W2:C_W2 + 8] = w2
    consts[0, C_B1:C_B1 + 64] = b1
    consts[0, C_B2:C_B2 + 8] = b2
    consts[0, C_ONES:C_ONES + 128] = 1.0
    mc = np.zeros((128, M_COLS), dtype=ml_dtypes.bfloat16)
    r = np.arange(128)
    for j in range(8):
        m = np.zeros((128, 128), dtype=np.float32)
        m[r, 16 * j + r // 8] = 1.0
        mc[:, M_MASK + j * 128:M_MASK + (j + 1) * 128] = m
    ks = np.zeros((8, 128), dtype=np.float32)
    ks[np.arange(8)[:, None] == np.arange(128)[None, :] % 8] = 1.0
    mc[0:8, M_KSEL:M_KSEL + 128] = ks
    return xh, bankh, consts, mc


def kernel(x, kernels, w1, b1, w2, b2):
    global _cached
    if _cached is None:
        _cached = _build()
    nc = _cached
    xh, bankh, consts, mc = _prep(np.asarray(x, dtype=np.float32),
                                  np.asarray(kernels, dtype=np.float32),
                                  np.asarray(w1, dtype=np.float32),
                                  np.asarray(b1, dtype=np.float32),
                                  np.asarray(w2, dtype=np.float32),
                                  np.asarray(b2, dtype=np.float32))
    in_maps = [{"x": xh[c * B:(c + 1) * B], "bank": bankh,
                "consts": consts, "mconsts": mc} for c in range(N_CORES)]
    res = run_bass_kernel_spmd(nc, in_maps, list(range(N_CORES)))
    out = np.concatenate(
        [res.results[c]["y"].reshape(B, C, H, W_IMG) for c in range(N_CORES)],
        axis=0)
    return out.astype(np.float32)


# revision 17
# speedup vs baseline: 1.0209x; 1.0209x over previous
"""DynamicConvolution Trainium2 kernel (8 NeuronCores, data-parallel over batch).

Reference computation (per sample b):
  pooled = mean(x[b], spatial); h = relu(pooled @ w1 + b1)
  alpha  = softmax(h @ w2 + b2)                   [8]
  W[b]   = sum_k alpha[k] * kernels[k]            [256,256,3,3]
  y[b]   = conv2d(x[b], W[b], pad=1)              [256,56,56]

Sharding: batch 16 -> 2 samples per core; bank + MLP weights replicated.

Device mapping (per core, bf16 compute, fp32 PSUM accumulation):
  - x arrives host-padded as [2,256,58*58] bf16 -> contiguous DMAs.
    DMA issue order x(b0), bank(o_t=0), x(b1), bank(o_t=1) minimizes the
    critical path: sample-0 attention runs while bank(o0) streams, so
    mixing matmuls start ~15us in.
  - attention is duplicated per sample (tiny) so alpha[b0] never waits on
    x[b1]: pooled via chunked VectorE reduces; MLP on TensorE with channels
    on partitions; biases as K=1 rank-1 matmuls; softmax VectorE/ScalarE;
    alphaT_b[128,1] (alphaT[c*8+k]=alpha[b,k]) via a K=8 selection matmul.
  - kernel mixing ON TensorE: per (b, i_tile, o_tile, tap-group) PSUM block
    [128i, g*128] accumulates 8 matmuls; matmul j uses a masked lhsT
    (alpha at ((c,k), 16j+c)) against the (c,k)-partition bank layout, so
    16 i-rows mix per streamed column.  VectorE evacuates straight into
    conv weight layout [i128, (s,o)] bf16.
  - conv: per (o_t, b, t) PSUM group of 18 accumulating matmuls
    [o128,448] += W[i128,o128]^T @ xpad[i128, 8x56 window]; ScalarE
    evacuates fp32; DMA out.

Sync discipline (walrus permits ONE semaphore wait per engine instruction):
  - matmul waits split: lhsT dep on InstLdweights, rhs dep on InstMatmult;
    PSUM-reuse WAR rides InstMatmult, so every matmul's rhs DMA queue is
    pre-observed by a 1x1 "touch" matmul, ordered with add_dep_helper.
  - mix-block PSUM evac on VectorE (conv LW waits merge on its sem);
    conv PSUM evac on ScalarE (conv MM WAR waits on its sem).
  - all tiny MLP/selection PSUM tiles share one bank tag sequentially; the
    touch scratch lives in its own bank with PE-local WAW only.
"""

import numpy as np
import ml_dtypes
from contextlib import ExitStack

try:
    import concourse.bass as bass
except ImportError:  # fresh grading dir: repo paths not on sys.path yet
    import sys
    for p in ("/opt/trn_rl_repo", "/root/.axon_site/_ro/trn_rl_repo"):
        if p not in sys.path:
            sys.path.append(p)
    import concourse.bass as bass

import concourse.mybir as mybir
import concourse.tile as tile
from concourse import bacc
from concourse.tile import add_dep_helper
from concourse.bass_utils import run_bass_kernel_spmd

F32 = mybir.dt.float32
BF16 = mybir.dt.bfloat16
AX = mybir.AxisListType.X
RELU = mybir.ActivationFunctionType.Relu
EXP = mybir.ActivationFunctionType.Exp
COPY = mybir.ActivationFunctionType.Copy

N_CORES = 8
B = 2               # samples per core
C = 256             # channels
IT = 2              # 128-channel input tiles
OT = 2              # 128-channel output tiles
H = W_IMG = 56
HW = H * W_IMG      # 3136
PADW = 58
PADHW = PADW * PADW  # 3364
NT = 7              # row blocks per image
TB = 448            # 8 rows x 56 cols per conv psum block
S = 9               # conv taps
WSL = S * 128       # 1152 = per (b,o_t,i_t) weight-slice elems
XCH = 4             # x DMA chunks per (b, i_tile)
XC = PADHW // XCH   # 841 elems per x chunk
SGRP = [(0, 4), (4, 4), (8, 1)]   # tap groups (start, len)

# mlp consts layout (fp32 [128, 336])
C_W1A, C_W1B, C_W2, C_B1, C_B2, C_ONES = 0, 64, 128, 136, 200, 208
C_COLS = 336
# mix consts layout (bf16 [128, 1152]): 8 masks | KSEL
M_MASK, M_KSEL = 0, 1024
M_COLS = 1152

_cached = None


def _build():
    nc = bacc.Bacc()
    xin = nc.declare_dram_parameter("x", [B, C, PADHW], BF16, isOutput=False)
    bankin = nc.declare_dram_parameter("bank", [128, OT * IT * 8 * WSL], BF16,
                                       isOutput=False)
    cin = nc.declare_dram_parameter("consts", [128, C_COLS], F32, isOutput=False)
    min_ = nc.declare_dram_parameter("mconsts", [128, M_COLS], BF16,
                                     isOutput=False)
    y = nc.declare_dram_parameter("y", [B, C, HW], F32, isOutput=True)

    with tile.TileContext(nc) as tc, ExitStack() as ctx:
        sb = ctx.enter_context(tc.tile_pool(name="sb", bufs=1))
        conv_ps = ctx.enter_context(tc.tile_pool(name="cps", bufs=4, space="PSUM"))
        mix_ps = ctx.enter_context(tc.tile_pool(name="xps", bufs=2, space="PSUM"))
        mlp_ps = ctx.enter_context(tc.tile_pool(name="mps", bufs=1, space="PSUM"))
        scr_ps = ctx.enter_context(tc.tile_pool(name="sps", bufs=1, space="PSUM"))

        xpad = sb.tile([128, B * IT * PADHW], BF16, tag="xpad")
        bank = sb.tile([128, OT * IT * 8 * WSL], BF16, tag="bank")
        wsb = sb.tile([128, B * OT * IT * WSL], BF16, tag="wsb")
        outsb = sb.tile([128, OT * B * NT * TB], F32, tag="outsb")
        consts = sb.tile([128, C_COLS], F32, tag="consts")
        mconsts = sb.tile([128, M_COLS], BF16, tag="mconsts")
        scratch = scr_ps.tile([1, 1], F32)

        def xv(b, it):
            base = (b * IT + it) * PADHW
            return xpad[:, base:base + PADHW].rearrange("p (r c) -> p r c", c=PADW)

        def pe_touch(ap):
            return nc.tensor.matmul(scratch[:], ap, ap, start=True, stop=True,
                                    skip_group_check=True)

        # ---------- DMAs: consts, then x(b0) | bank(o0) | x(b1) | bank(o1)
        nc.sync.dma_start(consts[:], cin[:])
        nc.sync.dma_start(mconsts[:], min_[:])

        def dma_x(b, eng=None, gate=None):
            eng = eng or nc.sync
            dmas = []
            for it in range(IT):
                base = (b * IT + it) * PADHW
                for cch in range(XCH):
                    d = eng.dma_start(
                        xpad[:, base + cch * XC: base + (cch + 1) * XC],
                        xin[b, it * 128:(it + 1) * 128,
                            cch * XC:(cch + 1) * XC])
                    if gate is not None:
                        add_dep_helper(d.ins, gate.ins, sync=False,
                                       reason="dma wave gate")
                    dmas.append(d)
            return dmas

        def dma_bank(ot, eng=None, gate=None):
            eng = eng or nc.sync
            dmas = []
            for it in range(IT):
                for jp in range(4):
                    off = (((ot * IT + it) * 8) + 2 * jp) * WSL
                    d = eng.dma_start(bank[:, off:off + 2 * WSL],
                                      bankin[:, off:off + 2 * WSL])
                    if gate is not None:
                        add_dep_helper(d.ins, gate.ins, sync=False,
                                       reason="dma wave gate")
                    dmas.append(d)
            return dmas

        def touch_x(b):
            ts = []
            for it in range(IT):
                base = (b * IT + it) * PADHW
                for cch in range(XCH):
                    ts.append(pe_touch(
                        xpad[0:1, base + cch * XC: base + cch * XC + 1]))
            return ts

        def touch_bank(ot):
            ts = []
            for it in range(IT):
                for jp in range(4):
                    off = (((ot * IT + it) * 8) + 2 * jp) * WSL
                    ts.append(pe_touch(bank[0:1, off:off + 1]))
            return ts

        # wave 1 on the SP HWDGE queue: x(b0) + bank(o0).
        # wave 2 on the ScalarE HWDGE queue, behind an ACT gate instruction
        # that waits for the last wave-1 bank slice: ScalarE's FIFO delays
        # the wave-2 descriptor enqueue until wave 1 has fully landed, so
        # wave 1 gets the whole HBM bandwidth first.
        dma_x(0)
        last_b0 = dma_bank(0)[-1]
        xtouch_b = [touch_x(0)]
        btch_o = [touch_bank(0)]
        gatebuf = sb.tile([1, 1], BF16, tag="gatebuf")
        act_gate = nc.scalar.activation(
            gatebuf[:], bank[0:1, IT * 8 * WSL - 1: IT * 8 * WSL], COPY)
        dma_x(1, eng=nc.scalar, gate=act_gate)
        dma_bank(1, eng=nc.scalar, gate=act_gate)
        xtouch_b.append(touch_x(1))
        btch_o.append(touch_bank(1))
        xtouch = xtouch_b[0] + btch_o[0] + xtouch_b[1] + btch_o[1]
        for t1, t0 in zip(xtouch[1:], xtouch[:-1]):
            add_dep_helper(t1.ins, t0.ins, sync=False, reason="touch chain")

        # ---------- per-sample attention -> alphaT -> masked lhsT tiles
        mtch = sb.tile([1, 1], BF16, tag="mtch")
        mtch_i = nc.vector.tensor_copy(mtch[:], mconsts[0:1, 0:1])
        partials = sb.tile([128, B * IT * XCH], F32, tag="partials")
        psum2 = sb.tile([128, B * IT], F32, tag="psum2")
        pooled = sb.tile([128, B * IT], F32, tag="pooled")   # col (b, it)
        lhsT = sb.tile([128, 16 * 128], BF16, tag="lhsT")    # (j, b) tiles

        for b in range(B):
            for it in range(IT):
                base = (b * IT + it) * PADHW
                for cch in range(XCH):
                    j = (b * IT + it) * XCH + cch
                    nc.vector.reduce_sum(
                        partials[:, j:j + 1],
                        xpad[:, base + cch * XC: base + (cch + 1) * XC], axis=AX)
            for it in range(IT):
                j = b * IT + it
                nc.vector.reduce_sum(psum2[:, j:j + 1],
                                     partials[:, j * XCH:(j + 1) * XCH], axis=AX)
                nc.vector.tensor_scalar_mul(pooled[:, j:j + 1],
                                            psum2[:, j:j + 1], 1.0 / HW)

            hT_ps = mlp_ps.tile([64, 1], F32, tag="mlp")
            nc.tensor.matmul(hT_ps[:], consts[0:1, C_B1:C_B1 + 64],
                             consts[0:1, C_ONES:C_ONES + 1],
                             start=True, stop=False)
            nc.tensor.matmul(hT_ps[:], consts[:, C_W1A:C_W1A + 64],
                             pooled[:, b * IT: b * IT + 1],
                             start=False, stop=False)
            nc.tensor.matmul(hT_ps[:], consts[:, C_W1B:C_W1B + 64],
                             pooled[:, b * IT + 1: b * IT + 2],
                             start=False, stop=True)
            hT = sb.tile([64, 1], F32, tag=f"hTs{b}")
            nc.scalar.activation(hT[:], hT_ps[:], RELU)

            sc_ps = mlp_ps.tile([1, 8], F32, tag="mlp")
            nc.tensor.matmul(sc_ps[:], consts[0:1, C_ONES:C_ONES + 1],
                             consts[0:1, C_B2:C_B2 + 8], start=True, stop=False)
            nc.tensor.matmul(sc_ps[:], hT[:], consts[0:64, C_W2:C_W2 + 8],
                             start=False, stop=True)

            scores = sb.tile([1, 8], F32, tag=f"scores{b}")
            nc.vector.tensor_copy(scores[:], sc_ps[:])
            mx = sb.tile([1, 1], F32, tag=f"mx{b}")
            nc.vector.reduce_max(mx[:], scores[:], axis=AX)
            subb = sb.tile([1, 8], F32, tag=f"subb{b}")
            nc.vector.tensor_scalar_sub(subb[:], scores[:], mx[:])
            ex = sb.tile([1, 8], F32, tag=f"ex{b}")
            nc.scalar.activation(ex[:], subb[:], EXP)
            z = sb.tile([1, 1], F32, tag=f"z{b}")
            nc.vector.reduce_sum(z[:], ex[:], axis=AX)
            rz = sb.tile([1, 1], F32, tag=f"rz{b}")
            nc.vector.reciprocal(rz[:], z[:])
            al = sb.tile([1, 8], F32, tag=f"al{b}")
            nc.vector.tensor_scalar_mul(al[:], ex[:], rz[:])

            a8 = sb.tile([8, 1], F32, tag=f"a8{b}")
            nc.sync.dma_start(a8[:], al[:])          # [1,8] -> [8,1]
            a8h = sb.tile([8, 1], BF16, tag=f"a8h{b}")
            a8h_i = nc.vector.tensor_copy(a8h[:], a8[:])
            if b == 0:
                add_dep_helper(a8h_i.ins, mtch_i.ins, sync=False,
                               reason="mconsts seen on DVE")
            aT_ps = mlp_ps.tile([128, 1], F32, tag="mlp")
            nc.tensor.matmul(aT_ps[:], mconsts[0:8, M_KSEL:M_KSEL + 128],
                             a8h[:], start=True, stop=True)
            aT = sb.tile([128, 1], F32, tag=f"aT{b}")
            nc.vector.tensor_copy(aT[:], aT_ps[:])
            for j in range(8):
                nc.vector.tensor_scalar_mul(
                    lhsT[:, (j * B + b) * 128:(j * B + b + 1) * 128],
                    mconsts[:, M_MASK + j * 128:M_MASK + (j + 1) * 128],
                    aT[:, 0:1])

        # ---------- per o_tile: PE mixing, conv
        for ot in range(OT):
            btch = btch_o[ot]

            # mixing: blocks (b, it, grp); 8 accumulated matmuls each
            for b in range(B):
                for it in range(IT):
                    woff = ((b * OT + ot) * IT + it) * WSL
                    for (s0, g) in SGRP:
                        mps = mix_ps.tile([128, 512], F32, tag="mix")
                        n = g * 128
                        for j in range(8):
                            boff = ((ot * IT + it) * 8 + j) * WSL + s0 * 128
                            mm = nc.tensor.matmul(
                                mps[:, 0:n],
                                lhsT[:, (j * B + b) * 128:(j * B + b + 1) * 128],
                                bank[:, boff:boff + n],
                                start=(j == 0), stop=(j == 7))
                            if j == 0:
                                add_dep_helper(mm.ins, btch[-1].ins, sync=False,
                                               reason="bank observed")
                        nc.vector.tensor_copy(
                            wsb[:, woff + s0 * 128: woff + (s0 + g) * 128],
                            mps[:, 0:n])

            # conv groups (b, t): 18 accumulated matmuls each
            for b in range(B):
                for t in range(NT):
                    ps = conv_ps.tile([128, TB], F32, tag="convps")
                    first_of_group = True
                    n_mm = 0
                    for it in range(IT):
                        woff = ((b * OT + ot) * IT + it) * WSL
                        v = xv(b, it)
                        for s in range(S):
                            kh, kw = s // 3, s % 3
                            mm = nc.tensor.matmul(
                                ps[:],
                                wsb[:, woff + s * 128: woff + (s + 1) * 128],
                                v[:, 8 * t + kh: 8 * t + kh + 8, kw:kw + 56],
                                start=(n_mm == 0), stop=(n_mm == 17))
                            if first_of_group and b == 1 and t == 0:
                                add_dep_helper(mm.ins, xtouch_b[1][-1].ins,
                                               sync=False, reason="x b1 seen")
                            first_of_group = False
                            n_mm += 1
                    blk = (ot * B + b) * NT + t
                    nc.scalar.activation(outsb[:, blk * TB:(blk + 1) * TB],
                                         ps[:], COPY)
                    nc.sync.dma_start(
                        y[b, ot * 128:(ot + 1) * 128, t * TB:(t + 1) * TB],
                        outsb[:, blk * TB:(blk + 1) * TB])

    nc.compile()
    return nc


def _prep(x, kernels, w1, b1, w2, b2):
    """Host-side marshaling: dtype casts + layout/padding rearrangement only."""
    xp = np.zeros((16, C, PADW, PADW), dtype=ml_dtypes.bfloat16)
    xp[:, :, 1:57, 1:57] = x
    xh = np.ascontiguousarray(xp.reshape(16, C, PADHW))
    # bank[(c,k), (ot, it, j, s, o)] = kernels[k, ot*128+o, it*128+16j+c, s]
    kr = kernels.reshape(8, OT, 128, IT, 8, 16, S)   # k,ot,o,it,j,c,s
    bankh = np.ascontiguousarray(
        kr.transpose(5, 0, 1, 3, 4, 6, 2).reshape(128, OT * IT * 8 * WSL)
        .astype(ml_dtypes.bfloat16))
    consts = np.zeros((128, C_COLS), dtype=np.float32)
    consts[:, C_W1A:C_W1A + 64] = w1[0:128]
    consts[:, C_W1B:C_W1B + 64] = w1[128:256]
    consts[0:64, C_W2:C_W2 + 8] = w2
    consts[0, C_B1:C_B1 + 64] = b1
    consts[0, C_B2:C_B2 + 8] = b2
    consts[0, C_ONES:C_ONES + 128] = 1.0
    mc = np.zeros((128, M_COLS), dtype=ml_dtypes.bfloat16)
    r = np.arange(128)
    for j in range(8):
        m = np.zeros((128, 128), dtype=np.float32)
        m[r, 16 * j + r // 8] = 1.0
        mc[:, M_MASK + j * 128:M_MASK + (j + 1) * 128] = m
    ks = np.zeros((8, 128), dtype=np.float32)
    ks[np.arange(8)[:, None] == np.arange(128)[None, :] % 8] = 1.0
    mc[0:8, M_KSEL:M_KSEL + 128] = ks
    return xh, bankh, consts, mc


def kernel(x, kernels, w1, b1, w2, b2):
    global _cached
    if _cached is None:
        _cached = _build()
    nc = _cached
    xh, bankh, consts, mc = _prep(np.asarray(x, dtype=np.float32),
                                  np.asarray(kernels, dtype=np.float32),
                                  np.asarray(w1, dtype=np.float32),
                                  np.asarray(b1, dtype=np.float32),
                                  np.asarray(w2, dtype=np.float32),
                                  np.asarray(b2, dtype=np.float32))
    in_maps = [{"x": xh[c * B:(c + 1) * B], "bank": bankh,
                "consts": consts, "mconsts": mc} for c in range(N_CORES)]
    res = run_bass_kernel_spmd(nc, in_maps, list(range(N_CORES)))
    out = np.concatenate(
        [res.results[c]["y"].reshape(B, C, H, W_IMG) for c in range(N_CORES)],
        axis=0)
    return out.astype(np.float32)


# revision 18
# speedup vs baseline: 1.0410x; 1.0198x over previous
"""DynamicConvolution Trainium2 kernel (8 NeuronCores, data-parallel over batch).

Reference computation (per sample b):
  pooled = mean(x[b], spatial); h = relu(pooled @ w1 + b1)
  alpha  = softmax(h @ w2 + b2)                   [8]
  W[b]   = sum_k alpha[k] * kernels[k]            [256,256,3,3]
  y[b]   = conv2d(x[b], W[b], pad=1)              [256,56,56]

Sharding: batch 16 -> 2 samples per core; bank + MLP weights replicated.

Device mapping (per core, bf16 compute, fp32 PSUM accumulation):
  - x arrives host-padded as [2,256,58*58] bf16 -> contiguous DMAs.
    DMA issue order x(b0), bank(o_t=0), x(b1), bank(o_t=1) minimizes the
    critical path: sample-0 attention runs while bank(o0) streams, so
    mixing matmuls start ~15us in.
  - attention is duplicated per sample (tiny) so alpha[b0] never waits on
    x[b1]: pooled via chunked VectorE reduces; MLP on TensorE with channels
    on partitions; biases as K=1 rank-1 matmuls; softmax VectorE/ScalarE;
    alphaT_b[128,1] (alphaT[c*8+k]=alpha[b,k]) via a K=8 selection matmul.
  - kernel mixing ON TensorE: per (b, i_tile, o_tile, tap-group) PSUM block
    [128i, g*128] accumulates 8 matmuls; matmul j uses a masked lhsT
    (alpha at ((c,k), 16j+c)) against the (c,k)-partition bank layout, so
    16 i-rows mix per streamed column.  VectorE evacuates straight into
    conv weight layout [i128, (s,o)] bf16.
  - conv: per (o_t, b, t) PSUM group of 18 accumulating matmuls
    [o128,448] += W[i128,o128]^T @ xpad[i128, 8x56 window]; ScalarE
    evacuates fp32; DMA out.

Sync discipline (walrus permits ONE semaphore wait per engine instruction):
  - matmul waits split: lhsT dep on InstLdweights, rhs dep on InstMatmult;
    PSUM-reuse WAR rides InstMatmult, so every matmul's rhs DMA queue is
    pre-observed by a 1x1 "touch" matmul, ordered with add_dep_helper.
  - mix-block PSUM evac on VectorE (conv LW waits merge on its sem);
    conv PSUM evac on ScalarE (conv MM WAR waits on its sem).
  - all tiny MLP/selection PSUM tiles share one bank tag sequentially; the
    touch scratch lives in its own bank with PE-local WAW only.
"""

import numpy as np
import ml_dtypes
from contextlib import ExitStack

try:
    import concourse.bass as bass
except ImportError:  # fresh grading dir: repo paths not on sys.path yet
    import sys
    for p in ("/opt/trn_rl_repo", "/root/.axon_site/_ro/trn_rl_repo"):
        if p not in sys.path:
            sys.path.append(p)
    import concourse.bass as bass

import concourse.mybir as mybir
import concourse.tile as tile
from concourse import bacc
from concourse.tile import add_dep_helper
from concourse.bass_utils import run_bass_kernel_spmd

F32 = mybir.dt.float32
BF16 = mybir.dt.bfloat16
AX = mybir.AxisListType.X
RELU = mybir.ActivationFunctionType.Relu
EXP = mybir.ActivationFunctionType.Exp
COPY = mybir.ActivationFunctionType.Copy

N_CORES = 8
B = 2               # samples per core
C = 256             # channels
IT = 2              # 128-channel input tiles
OT = 2              # 128-channel output tiles
H = W_IMG = 56
HW = H * W_IMG      # 3136
PADW = 58
PADHW = PADW * PADW  # 3364
NT = 7              # row blocks per image
TB = 448            # 8 rows x 56 cols per conv psum block
S = 9               # conv taps
WSL = S * 128       # 1152 = per (b,o_t,i_t) weight-slice elems
XCH = 4             # x DMA chunks per (b, i_tile)
XC = PADHW // XCH   # 841 elems per x chunk
SGRP = [(0, 4), (4, 4), (8, 1)]   # tap groups (start, len)

# mlp consts layout (fp32 [128, 336])
C_W1A, C_W1B, C_W2, C_B1, C_B2, C_ONES = 0, 64, 128, 136, 200, 208
C_COLS = 336
# mix consts layout (bf16 [128, 1152]): 8 masks | KSEL
M_MASK, M_KSEL = 0, 1024
M_COLS = 1152

_cached = None


def _build():
    nc = bacc.Bacc()
    xin = nc.declare_dram_parameter("x", [B, C, PADHW], BF16, isOutput=False)
    bankin = nc.declare_dram_parameter("bank", [128, OT * IT * 8 * WSL], BF16,
                                       isOutput=False)
    cin = nc.declare_dram_parameter("consts", [128, C_COLS], F32, isOutput=False)
    min_ = nc.declare_dram_parameter("mconsts", [128, M_COLS], BF16,
                                     isOutput=False)
    y = nc.declare_dram_parameter("y", [B, C, HW], F32, isOutput=True)

    with tile.TileContext(nc) as tc, ExitStack() as ctx:
        sb = ctx.enter_context(tc.tile_pool(name="sb", bufs=1))
        conv_ps = ctx.enter_context(tc.tile_pool(name="cps", bufs=4, space="PSUM"))
        mix_ps = ctx.enter_context(tc.tile_pool(name="xps", bufs=2, space="PSUM"))
        mlp_ps = ctx.enter_context(tc.tile_pool(name="mps", bufs=1, space="PSUM"))
        scr_ps = ctx.enter_context(tc.tile_pool(name="sps", bufs=1, space="PSUM"))

        xpad = sb.tile([128, B * IT * PADHW], BF16, tag="xpad")
        bank = sb.tile([128, OT * IT * 8 * WSL], BF16, tag="bank")
        wsb = sb.tile([128, B * OT * IT * WSL], BF16, tag="wsb")
        outsb = sb.tile([128, OT * B * NT * TB], F32, tag="outsb")
        consts = sb.tile([128, C_COLS], F32, tag="consts")
        mconsts = sb.tile([128, M_COLS], BF16, tag="mconsts")
        scratch = scr_ps.tile([1, 1], F32)

        def xv(b, it):
            base = (b * IT + it) * PADHW
            return xpad[:, base:base + PADHW].rearrange("p (r c) -> p r c", c=PADW)

        def pe_touch(ap):
            return nc.tensor.matmul(scratch[:], ap, ap, start=True, stop=True,
                                    skip_group_check=True)

        # ---------- DMAs: consts, then x(b0) | bank(o0) | x(b1) | bank(o1)
        nc.sync.dma_start(consts[:], cin[:])
        nc.sync.dma_start(mconsts[:], min_[:])

        def dma_x(b, eng=None, gate=None):
            eng = eng or nc.sync
            dmas = []
            for it in range(IT):
                base = (b * IT + it) * PADHW
                for cch in range(XCH):
                    d = eng.dma_start(
                        xpad[:, base + cch * XC: base + (cch + 1) * XC],
                        xin[b, it * 128:(it + 1) * 128,
                            cch * XC:(cch + 1) * XC])
                    if gate is not None:
                        add_dep_helper(d.ins, gate.ins, sync=False,
                                       reason="dma wave gate")
                    dmas.append(d)
            return dmas

        def dma_bank(ot, eng=None, gate=None):
            eng = eng or nc.sync
            dmas = []
            for it in range(IT):
                for jp in range(4):
                    off = (((ot * IT + it) * 8) + 2 * jp) * WSL
                    d = eng.dma_start(bank[:, off:off + 2 * WSL],
                                      bankin[:, off:off + 2 * WSL])
                    if gate is not None:
                        add_dep_helper(d.ins, gate.ins, sync=False,
                                       reason="dma wave gate")
                    dmas.append(d)
            return dmas

        def touch_x(b):
            ts = []
            for it in range(IT):
                base = (b * IT + it) * PADHW
                for cch in range(XCH):
                    ts.append(pe_touch(
                        xpad[0:1, base + cch * XC: base + cch * XC + 1]))
            return ts

        def touch_bank(ot):
            ts = []
            for it in range(IT):
                for jp in range(4):
                    off = (((ot * IT + it) * 8) + 2 * jp) * WSL
                    ts.append(pe_touch(bank[0:1, off:off + 1]))
            return ts

        # ---------- shared small tiles
        mtch = sb.tile([1, 1], BF16, tag="mtch")
        partials = sb.tile([128, B * IT * XCH], F32, tag="partials")
        psum2 = sb.tile([128, B * IT], F32, tag="psum2")
        pooled = sb.tile([128, B * IT], F32, tag="pooled")   # col (b, it)
        lhsT = sb.tile([128, 16 * 128], BF16, tag="lhsT")    # (j, b) tiles

        def attention(b):
            """pooled -> MLP -> softmax -> alphaT -> masked lhsT tiles for b.
            Returns the ScalarE exp instruction (ACT-order anchor)."""
            for it in range(IT):
                base = (b * IT + it) * PADHW
                for cch in range(XCH):
                    j = (b * IT + it) * XCH + cch
                    nc.vector.reduce_sum(
                        partials[:, j:j + 1],
                        xpad[:, base + cch * XC: base + (cch + 1) * XC], axis=AX)
            for it in range(IT):
                j = b * IT + it
                nc.vector.reduce_sum(psum2[:, j:j + 1],
                                     partials[:, j * XCH:(j + 1) * XCH], axis=AX)
                nc.vector.tensor_scalar_mul(pooled[:, j:j + 1],
                                            psum2[:, j:j + 1], 1.0 / HW)

            hT_ps = mlp_ps.tile([64, 1], F32, tag="mlp")
            nc.tensor.matmul(hT_ps[:], consts[0:1, C_B1:C_B1 + 64],
                             consts[0:1, C_ONES:C_ONES + 1],
                             start=True, stop=False)
            nc.tensor.matmul(hT_ps[:], consts[:, C_W1A:C_W1A + 64],
                             pooled[:, b * IT: b * IT + 1],
                             start=False, stop=False)
            nc.tensor.matmul(hT_ps[:], consts[:, C_W1B:C_W1B + 64],
                             pooled[:, b * IT + 1: b * IT + 2],
                             start=False, stop=True)
            hT = sb.tile([64, 1], F32, tag=f"hTs{b}")
            nc.scalar.activation(hT[:], hT_ps[:], RELU)

            sc_ps = mlp_ps.tile([1, 8], F32, tag="mlp")
            nc.tensor.matmul(sc_ps[:], consts[0:1, C_ONES:C_ONES + 1],
                             consts[0:1, C_B2:C_B2 + 8], start=True, stop=False)
            nc.tensor.matmul(sc_ps[:], hT[:], consts[0:64, C_W2:C_W2 + 8],
                             start=False, stop=True)

            scores = sb.tile([1, 8], F32, tag=f"scores{b}")
            nc.vector.tensor_copy(scores[:], sc_ps[:])
            mx = sb.tile([1, 1], F32, tag=f"mx{b}")
            nc.vector.reduce_max(mx[:], scores[:], axis=AX)
            subb = sb.tile([1, 8], F32, tag=f"subb{b}")
            nc.vector.tensor_scalar_sub(subb[:], scores[:], mx[:])
            ex = sb.tile([1, 8], F32, tag=f"ex{b}")
            exp_i = nc.scalar.activation(ex[:], subb[:], EXP)
            z = sb.tile([1, 1], F32, tag=f"z{b}")
            nc.vector.reduce_sum(z[:], ex[:], axis=AX)
            rz = sb.tile([1, 1], F32, tag=f"rz{b}")
            nc.vector.reciprocal(rz[:], z[:])
            al = sb.tile([1, 8], F32, tag=f"al{b}")
            nc.vector.tensor_scalar_mul(al[:], ex[:], rz[:])

            a8 = sb.tile([8, 1], F32, tag=f"a8{b}")
            nc.sync.dma_start(a8[:], al[:])          # [1,8] -> [8,1]
            a8h = sb.tile([8, 1], BF16, tag=f"a8h{b}")
            a8h_i = nc.vector.tensor_copy(a8h[:], a8[:])
            if b == 0:
                mtch_i = nc.vector.tensor_copy(mtch[:], mconsts[0:1, 0:1])
                add_dep_helper(a8h_i.ins, mtch_i.ins, sync=False,
                               reason="mconsts seen on DVE")
            aT_ps = mlp_ps.tile([128, 1], F32, tag="mlp")
            nc.tensor.matmul(aT_ps[:], mconsts[0:8, M_KSEL:M_KSEL + 128],
                             a8h[:], start=True, stop=True)
            aT = sb.tile([128, 1], F32, tag=f"aT{b}")
            nc.vector.tensor_copy(aT[:], aT_ps[:])
            for j in range(8):
                nc.vector.tensor_scalar_mul(
                    lhsT[:, (j * B + b) * 128:(j * B + b + 1) * 128],
                    mconsts[:, M_MASK + j * 128:M_MASK + (j + 1) * 128],
                    aT[:, 0:1])
            return exp_i

        def mix(ot, b, btch_last):
            for it in range(IT):
                woff = ((b * OT + ot) * IT + it) * WSL
                for (s0, g) in SGRP:
                    mps = mix_ps.tile([128, 512], F32, tag="mix")
                    n = g * 128
                    for j in range(8):
                        boff = ((ot * IT + it) * 8 + j) * WSL + s0 * 128
                        mm = nc.tensor.matmul(
                            mps[:, 0:n],
                            lhsT[:, (j * B + b) * 128:(j * B + b + 1) * 128],
                            bank[:, boff:boff + n],
                            start=(j == 0), stop=(j == 7))
                        if j == 0:
                            add_dep_helper(mm.ins, btch_last.ins, sync=False,
                                           reason="bank observed")
                    nc.vector.tensor_copy(
                        wsb[:, woff + s0 * 128: woff + (s0 + g) * 128],
                        mps[:, 0:n])

        def conv(ot, b, xtouch_last):
            for t in range(NT):
                ps = conv_ps.tile([128, TB], F32, tag="convps")
                n_mm = 0
                for it in range(IT):
                    woff = ((b * OT + ot) * IT + it) * WSL
                    v = xv(b, it)
                    for s in range(S):
                        kh, kw = s // 3, s % 3
                        mm = nc.tensor.matmul(
                            ps[:],
                            wsb[:, woff + s * 128: woff + (s + 1) * 128],
                            v[:, 8 * t + kh: 8 * t + kh + 8, kw:kw + 56],
                            start=(n_mm == 0), stop=(n_mm == 17))
                        if n_mm == 0 and t == 0 and xtouch_last is not None:
                            add_dep_helper(mm.ins, xtouch_last.ins, sync=False,
                                           reason="xpad observed")
                        n_mm += 1
                blk = (ot * B + b) * NT + t
                nc.scalar.activation(outsb[:, blk * TB:(blk + 1) * TB],
                                     ps[:], COPY)
                nc.sync.dma_start(
                    y[b, ot * 128:(ot + 1) * 128, t * TB:(t + 1) * TB],
                    outsb[:, blk * TB:(blk + 1) * TB])

        def chain(ts, prev=None):
            if prev is not None and ts:
                add_dep_helper(ts[0].ins, prev.ins, sync=False, reason="chain")
            for t1, t0 in zip(ts[1:], ts[:-1]):
                add_dep_helper(t1.ins, t0.ins, sync=False, reason="chain")
            return ts

        # ---------- emission in intended runtime order ----------
        # wave 1 on the SP HWDGE queue: x(b0) + bank(o0)
        dma_x(0)
        dma_bank(0)
        xt0 = chain(touch_x(0))
        bt0 = chain(touch_bank(0), prev=xt0[-1])
        exp_b0 = attention(0)

        # wave 2 on the ScalarE HWDGE queue, behind an ACT gate that waits
        # for the last wave-1 bank slice: ScalarE's FIFO delays the wave-2
        # descriptor enqueue until wave 1 has fully landed.
        gatebuf = sb.tile([1, 1], BF16, tag="gatebuf")
        act_gate = nc.scalar.activation(
            gatebuf[:], bank[0:1, IT * 8 * WSL - 1: IT * 8 * WSL], COPY)
        add_dep_helper(act_gate.ins, exp_b0.ins, sync=False,
                       reason="b0 softmax before gate on ACT")
        dma_x(1, eng=nc.scalar, gate=act_gate)
        dma_bank(1, eng=nc.scalar, gate=act_gate)

        mix(0, 0, bt0[-1])
        conv(0, 0, xt0[-1])

        xt1 = chain(touch_x(1), prev=bt0[-1])
        attention(1)
        mix(0, 1, bt0[-1])
        conv(0, 1, xt1[-1])

        bt1 = chain(touch_bank(1), prev=xt1[-1])
        mix(1, 0, bt1[-1])
        conv(1, 0, None)
        mix(1, 1, bt1[-1])
        conv(1, 1, None)

    nc.compile()
    return nc


def _prep(x, kernels, w1, b1, w2, b2):
    """Host-side marshaling: dtype casts + layout/padding rearrangement only."""
    xp = np.zeros((16, C, PADW, PADW), dtype=ml_dtypes.bfloat16)
    xp[:, :, 1:57, 1:57] = x
    xh = np.ascontiguousarray(xp.reshape(16, C, PADHW))
    # bank[(c,k), (ot, it, j, s, o)] = kernels[k, ot*128+o, it*128+16j+c, s]
    kr = kernels.reshape(8, OT, 128, IT, 8, 16, S)   # k,ot,o,it,j,c,s
    bankh = np.ascontiguousarray(
        kr.transpose(5, 0, 1, 3, 4, 6, 2).reshape(128, OT * IT * 8 * WSL)
        .astype(ml_dtypes.bfloat16))
    consts = np.zeros((128, C_COLS), dtype=np.float32)
    consts[:, C_W1A:C_W1A + 64] = w1[0:128]
    consts[:, C_W1B:C_W1B + 64] = w1[128:256]
    consts[0:64, C_W2:C_W2 + 8] = w2
    consts[0, C_B1:C_B1 + 64] = b1
    consts[0, C_B2:C_B2 + 8] = b2
    consts[0, C_ONES:C_ONES + 128] = 1.0
    mc = np.zeros((128, M_COLS), dtype=ml_dtypes.bfloat16)
    r = np.arange(128)
    for j in range(8):
        m = np.zeros((128, 128), dtype=np.float32)
        m[r, 16 * j + r // 8] = 1.0
        mc[:, M_MASK + j * 128:M_MASK + (j + 1) * 128] = m
    ks = np.zeros((8, 128), dtype=np.float32)
    ks[np.arange(8)[:, None] == np.arange(128)[None, :] % 8] = 1.0
    mc[0:8, M_KSEL:M_KSEL + 128] = ks
    return xh, bankh, consts, mc


def kernel(x, kernels, w1, b1, w2, b2):
    global _cached
    if _cached is None:
        _cached = _build()
    nc = _cached
    xh, bankh, consts, mc = _prep(np.asarray(x, dtype=np.float32),
                                  np.asarray(kernels, dtype=np.float32),
                                  np.asarray(w1, dtype=np.float32),
                                  np.asarray(b1, dtype=np.float32),
                                  np.asarray(w2, dtype=np.float32),
                                  np.asarray(b2, dtype=np.float32))
    in_maps = [{"x": xh[c * B:(c + 1) * B], "bank": bankh,
                "consts": consts, "mconsts": mc} for c in range(N_CORES)]
    res = run_bass_kernel_spmd(nc, in_maps, list(range(N_CORES)))
    out = np.concatenate(
        [res.results[c]["y"].reshape(B, C, H, W_IMG) for c in range(N_CORES)],
        axis=0)
    return out.astype(np.float32)


# revision 19
# speedup vs baseline: 1.0893x; 1.0463x over previous
"""DynamicConvolution Trainium2 kernel (8 NeuronCores, data-parallel over batch).

Reference computation (per sample b):
  pooled = mean(x[b], spatial); h = relu(pooled @ w1 + b1)
  alpha  = softmax(h @ w2 + b2)                   [8]
  W[b]   = sum_k alpha[k] * kernels[k]            [256,256,3,3]
  y[b]   = conv2d(x[b], W[b], pad=1)              [256,56,56]

Sharding: batch 16 -> 2 samples per core; bank + MLP weights replicated.

Device mapping (per core, bf16 compute, fp32 PSUM accumulation):
  - x arrives host-padded as [2,256,58*58] bf16 -> contiguous DMAs.
    DMA issue order x(b0), bank(o_t=0), x(b1), bank(o_t=1) minimizes the
    critical path: sample-0 attention runs while bank(o0) streams, so
    mixing matmuls start ~15us in.
  - attention is duplicated per sample (tiny) so alpha[b0] never waits on
    x[b1]: pooled via chunked VectorE reduces; MLP on TensorE with channels
    on partitions; biases as K=1 rank-1 matmuls; softmax VectorE/ScalarE;
    alphaT_b[128,1] (alphaT[c*8+k]=alpha[b,k]) via a K=8 selection matmul.
  - kernel mixing ON TensorE: per (b, i_tile, o_tile, tap-group) PSUM block
    [128i, g*128] accumulates 8 matmuls; matmul j uses a masked lhsT
    (alpha at ((c,k), 16j+c)) against the (c,k)-partition bank layout, so
    16 i-rows mix per streamed column.  VectorE evacuates straight into
    conv weight layout [i128, (s,o)] bf16.
  - conv: per (o_t, b, t) PSUM group of 18 accumulating matmuls
    [o128,448] += W[i128,o128]^T @ xpad[i128, 8x56 window]; ScalarE
    evacuates fp32; DMA out.

Sync discipline (walrus permits ONE semaphore wait per engine instruction):
  - matmul waits split: lhsT dep on InstLdweights, rhs dep on InstMatmult;
    PSUM-reuse WAR rides InstMatmult, so every matmul's rhs DMA queue is
    pre-observed by a 1x1 "touch" matmul, ordered with add_dep_helper.
  - mix-block PSUM evac on VectorE (conv LW waits merge on its sem);
    conv PSUM evac on ScalarE (conv MM WAR waits on its sem).
  - all tiny MLP/selection PSUM tiles share one bank tag sequentially; the
    touch scratch lives in its own bank with PE-local WAW only.
"""

import numpy as np
import ml_dtypes
from contextlib import ExitStack

try:
    import concourse.bass as bass
except ImportError:  # fresh grading dir: repo paths not on sys.path yet
    import sys
    for p in ("/opt/trn_rl_repo", "/root/.axon_site/_ro/trn_rl_repo"):
        if p not in sys.path:
            sys.path.append(p)
    import concourse.bass as bass

import concourse.mybir as mybir
import concourse.tile as tile
from concourse import bacc
from concourse.tile import add_dep_helper
from concourse.bass_utils import run_bass_kernel_spmd

F32 = mybir.dt.float32
BF16 = mybir.dt.bfloat16
AX = mybir.AxisListType.X
RELU = mybir.ActivationFunctionType.Relu
EXP = mybir.ActivationFunctionType.Exp
COPY = mybir.ActivationFunctionType.Copy

N_CORES = 8
B = 2               # samples per core
C = 256             # channels
IT = 2              # 128-channel input tiles
OT = 2              # 128-channel output tiles
H = W_IMG = 56
HW = H * W_IMG      # 3136
PADW = 58
PADHW = PADW * PADW  # 3364
NT = 7              # row blocks per image
TB = 448            # 8 rows x 56 cols per conv psum block
S = 9               # conv taps
WSL = S * 128       # 1152 = per (b,o_t,i_t) weight-slice elems
XCH = 4             # x DMA chunks per (b, i_tile)
XC = PADHW // XCH   # 841 elems per x chunk
SGRP = [(0, 4), (4, 4), (8, 1)]   # tap groups (start, len)

# mlp consts layout (fp32 [128, 336])
C_W1A, C_W1B, C_W2, C_B1, C_B2, C_ONES = 0, 64, 128, 136, 200, 208
C_COLS = 336
# mix consts layout (bf16 [128, 1152]): 8 masks | KSEL
M_MASK, M_KSEL = 0, 1024
M_COLS = 1152

_cached = None


def _build():
    nc = bacc.Bacc()
    xin = nc.declare_dram_parameter("x", [B, C, PADHW], BF16, isOutput=False)
    bankin = nc.declare_dram_parameter("bank", [128, OT * IT * 8 * WSL], BF16,
                                       isOutput=False)
    cin = nc.declare_dram_parameter("consts", [128, C_COLS], F32, isOutput=False)
    min_ = nc.declare_dram_parameter("mconsts", [128, M_COLS], BF16,
                                     isOutput=False)
    y = nc.declare_dram_parameter("y", [B, C, HW], F32, isOutput=True)

    with tile.TileContext(nc) as tc, ExitStack() as ctx:
        sb = ctx.enter_context(tc.tile_pool(name="sb", bufs=1))
        conv_ps = ctx.enter_context(tc.tile_pool(name="cps", bufs=4, space="PSUM"))
        mix_ps = ctx.enter_context(tc.tile_pool(name="xps", bufs=2, space="PSUM"))
        mlp_ps = ctx.enter_context(tc.tile_pool(name="mps", bufs=1, space="PSUM"))
        scr_ps = ctx.enter_context(tc.tile_pool(name="sps", bufs=1, space="PSUM"))

        xpad = sb.tile([128, B * IT * PADHW], BF16, tag="xpad")
        bank = sb.tile([128, OT * IT * 8 * WSL], BF16, tag="bank")
        wsb = sb.tile([128, B * OT * IT * WSL], BF16, tag="wsb")
        outsb = sb.tile([128, OT * B * NT * TB], F32, tag="outsb")
        consts = sb.tile([128, C_COLS], F32, tag="consts")
        mconsts = sb.tile([128, M_COLS], BF16, tag="mconsts")
        scratch = scr_ps.tile([1, 1], F32)

        def xv(b, it):
            base = (b * IT + it) * PADHW
            return xpad[:, base:base + PADHW].rearrange("p (r c) -> p r c", c=PADW)

        def pe_touch(ap):
            return nc.tensor.matmul(scratch[:], ap, ap, start=True, stop=True,
                                    skip_group_check=True)

        # ---------- DMAs: consts, then x(b0) | bank(o0) | x(b1) | bank(o1)
        nc.sync.dma_start(consts[:], cin[:])
        nc.sync.dma_start(mconsts[:], min_[:])

        def dma_x(b, eng=None, gate=None):
            eng = eng or nc.sync
            dmas = []
            for it in range(IT):
                base = (b * IT + it) * PADHW
                d = eng.dma_start(xpad[:, base:base + PADHW],
                                  xin[b, it * 128:(it + 1) * 128, :])
                if gate is not None:
                    add_dep_helper(d.ins, gate.ins, sync=False,
                                   reason="dma wave gate")
                dmas.append(d)
            return dmas

        def dma_bank(ot, eng=None, gate=None):
            eng = eng or nc.sync
            dmas = []
            for it in range(IT):
                off = (ot * IT + it) * 8 * WSL
                d = eng.dma_start(bank[:, off:off + 8 * WSL],
                                  bankin[:, off:off + 8 * WSL])
                if gate is not None:
                    add_dep_helper(d.ins, gate.ins, sync=False,
                                   reason="dma wave gate")
                dmas.append(d)
            return dmas

        def touch_x(b):
            ts = []
            for it in range(IT):
                base = (b * IT + it) * PADHW
                ts.append(pe_touch(xpad[0:1, base: base + 1]))
            return ts

        def touch_bank(ot):
            ts = []
            for it in range(IT):
                off = (ot * IT + it) * 8 * WSL
                ts.append(pe_touch(bank[0:1, off:off + 1]))
            return ts

        # ---------- shared small tiles
        mtch = sb.tile([1, 1], BF16, tag="mtch")
        partials = sb.tile([128, B * IT * XCH], F32, tag="partials")
        psum2 = sb.tile([128, B * IT], F32, tag="psum2")
        pooled = sb.tile([128, B * IT], F32, tag="pooled")   # col (b, it)
        lhsT = sb.tile([128, 16 * 128], BF16, tag="lhsT")    # (j, b) tiles

        def attention(b):
            """pooled -> MLP -> softmax -> alphaT -> masked lhsT tiles for b.
            Returns the ScalarE exp instruction (ACT-order anchor)."""
            for it in range(IT):
                base = (b * IT + it) * PADHW
                for cch in range(XCH):
                    j = (b * IT + it) * XCH + cch
                    nc.vector.reduce_sum(
                        partials[:, j:j + 1],
                        xpad[:, base + cch * XC: base + (cch + 1) * XC], axis=AX)
            for it in range(IT):
                j = b * IT + it
                nc.vector.reduce_sum(psum2[:, j:j + 1],
                                     partials[:, j * XCH:(j + 1) * XCH], axis=AX)
                nc.vector.tensor_scalar_mul(pooled[:, j:j + 1],
                                            psum2[:, j:j + 1], 1.0 / HW)

            hT_ps = mlp_ps.tile([64, 1], F32, tag="mlp")
            nc.tensor.matmul(hT_ps[:], consts[0:1, C_B1:C_B1 + 64],
                             consts[0:1, C_ONES:C_ONES + 1],
                             start=True, stop=False)
            nc.tensor.matmul(hT_ps[:], consts[:, C_W1A:C_W1A + 64],
                             pooled[:, b * IT: b * IT + 1],
                             start=False, stop=False)
            nc.tensor.matmul(hT_ps[:], consts[:, C_W1B:C_W1B + 64],
                             pooled[:, b * IT + 1: b * IT + 2],
                             start=False, stop=True)
            hT = sb.tile([64, 1], F32, tag=f"hTs{b}")
            nc.scalar.activation(hT[:], hT_ps[:], RELU)

            sc_ps = mlp_ps.tile([1, 8], F32, tag="mlp")
            nc.tensor.matmul(sc_ps[:], consts[0:1, C_ONES:C_ONES + 1],
                             consts[0:1, C_B2:C_B2 + 8], start=True, stop=False)
            nc.tensor.matmul(sc_ps[:], hT[:], consts[0:64, C_W2:C_W2 + 8],
                             start=False, stop=True)

            scores = sb.tile([1, 8], F32, tag=f"scores{b}")
            nc.vector.tensor_copy(scores[:], sc_ps[:])
            mx = sb.tile([1, 1], F32, tag=f"mx{b}")
            nc.vector.reduce_max(mx[:], scores[:], axis=AX)
            subb = sb.tile([1, 8], F32, tag=f"subb{b}")
            nc.vector.tensor_scalar_sub(subb[:], scores[:], mx[:])
            ex = sb.tile([1, 8], F32, tag=f"ex{b}")
            exp_i = nc.scalar.activation(ex[:], subb[:], EXP)
            z = sb.tile([1, 1], F32, tag=f"z{b}")
            nc.vector.reduce_sum(z[:], ex[:], axis=AX)
            rz = sb.tile([1, 1], F32, tag=f"rz{b}")
            nc.vector.reciprocal(rz[:], z[:])
            al = sb.tile([1, 8], F32, tag=f"al{b}")
            nc.vector.tensor_scalar_mul(al[:], ex[:], rz[:])

            a8 = sb.tile([8, 1], F32, tag=f"a8{b}")
            nc.sync.dma_start(a8[:], al[:])          # [1,8] -> [8,1]
            a8h = sb.tile([8, 1], BF16, tag=f"a8h{b}")
            a8h_i = nc.vector.tensor_copy(a8h[:], a8[:])
            if b == 0:
                mtch_i = nc.vector.tensor_copy(mtch[:], mconsts[0:1, 0:1])
                add_dep_helper(a8h_i.ins, mtch_i.ins, sync=False,
                               reason="mconsts seen on DVE")
            aT_ps = mlp_ps.tile([128, 1], F32, tag="mlp")
            nc.tensor.matmul(aT_ps[:], mconsts[0:8, M_KSEL:M_KSEL + 128],
                             a8h[:], start=True, stop=True)
            aT = sb.tile([128, 1], F32, tag=f"aT{b}")
            nc.vector.tensor_copy(aT[:], aT_ps[:])
            for j in range(8):
                nc.vector.tensor_scalar_mul(
                    lhsT[:, (j * B + b) * 128:(j * B + b + 1) * 128],
                    mconsts[:, M_MASK + j * 128:M_MASK + (j + 1) * 128],
                    aT[:, 0:1])
            return exp_i

        def mix(ot, b, btch_last):
            for it in range(IT):
                woff = ((b * OT + ot) * IT + it) * WSL
                for (s0, g) in SGRP:
                    mps = mix_ps.tile([128, 512], F32, tag="mix")
                    n = g * 128
                    for j in range(8):
                        boff = ((ot * IT + it) * 8 + j) * WSL + s0 * 128
                        mm = nc.tensor.matmul(
                            mps[:, 0:n],
                            lhsT[:, (j * B + b) * 128:(j * B + b + 1) * 128],
                            bank[:, boff:boff + n],
                            start=(j == 0), stop=(j == 7))
                        if j == 0:
                            add_dep_helper(mm.ins, btch_last.ins, sync=False,
                                           reason="bank observed")
                    nc.vector.tensor_copy(
                        wsb[:, woff + s0 * 128: woff + (s0 + g) * 128],
                        mps[:, 0:n])

        def conv(ot, b, xtouch_last, ts=None):
            for t in (range(NT) if ts is None else ts):
                ps = conv_ps.tile([128, TB], F32, tag="convps")
                n_mm = 0
                for it in range(IT):
                    woff = ((b * OT + ot) * IT + it) * WSL
                    v = xv(b, it)
                    for s in range(S):
                        kh, kw = s // 3, s % 3
                        mm = nc.tensor.matmul(
                            ps[:],
                            wsb[:, woff + s * 128: woff + (s + 1) * 128],
                            v[:, 8 * t + kh: 8 * t + kh + 8, kw:kw + 56],
                            start=(n_mm == 0), stop=(n_mm == 17))
                        if n_mm == 0 and t == (ts[0] if ts else 0) \
                                and xtouch_last is not None:
                            add_dep_helper(mm.ins, xtouch_last.ins, sync=False,
                                           reason="xpad observed")
                        n_mm += 1
                blk = (ot * B + b) * NT + t
                nc.scalar.activation(outsb[:, blk * TB:(blk + 1) * TB],
                                     ps[:], COPY)
                nc.sync.dma_start(
                    y[b, ot * 128:(ot + 1) * 128, t * TB:(t + 1) * TB],
                    outsb[:, blk * TB:(blk + 1) * TB])

        def chain(ts, prev=None):
            if prev is not None and ts:
                add_dep_helper(ts[0].ins, prev.ins, sync=False, reason="chain")
            for t1, t0 in zip(ts[1:], ts[:-1]):
                add_dep_helper(t1.ins, t0.ins, sync=False, reason="chain")
            return ts

        # ---------- emission in intended runtime order ----------
        # wave 1 on the SP HWDGE queue: x(b0) + bank(o0)
        dma_x(0)
        dma_bank(0)
        xt0 = chain(touch_x(0))
        bt0 = chain(touch_bank(0), prev=xt0[-1])
        exp_b0 = attention(0)

        # wave 2 on the ScalarE HWDGE queue, behind an ACT gate that waits
        # for the last wave-1 bank slice: ScalarE's FIFO delays the wave-2
        # descriptor enqueue until wave 1 has fully landed.
        gatebuf = sb.tile([1, 1], BF16, tag="gatebuf")
        act_gate = nc.scalar.activation(
            gatebuf[:], bank[0:1, IT * 8 * WSL - 1: IT * 8 * WSL], COPY)
        add_dep_helper(act_gate.ins, exp_b0.ins, sync=False,
                       reason="b0 softmax before gate on ACT")
        dma_x(1, eng=nc.scalar, gate=act_gate)
        dma_bank(1, eng=nc.scalar, gate=act_gate)

        mix(0, 0, bt0[-1])
        conv(0, 0, xt0[-1], ts=[0, 1, 2])
        xt1 = chain(touch_x(1), prev=bt0[-1])
        attention(1)
        conv(0, 0, None, ts=[3, 4, 5, 6])
        mix(0, 1, bt0[-1])
        conv(0, 1, xt1[-1])

        bt1 = chain(touch_bank(1), prev=xt1[-1])
        mix(1, 0, bt1[-1])
        conv(1, 0, None)
        mix(1, 1, bt1[-1])
        conv(1, 1, None)

    nc.compile()
    return nc


def _prep(x, kernels, w1, b1, w2, b2):
    """Host-side marshaling: dtype casts + layout/padding rearrangement only."""
    xp = np.zeros((16, C, PADW, PADW), dtype=ml_dtypes.bfloat16)
    xp[:, :, 1:57, 1:57] = x
    xh = np.ascontiguousarray(xp.reshape(16, C, PADHW))
    # bank[(c,k), (ot, it, j, s, o)] = kernels[k, ot*128+o, it*128+16j+c, s]
    kr = kernels.reshape(8, OT, 128, IT, 8, 16, S)   # k,ot,o,it,j,c,s
    bankh = np.ascontiguousarray(
        kr.transpose(5, 0, 1, 3, 4, 6, 2).reshape(128, OT * IT * 8 * WSL)
        .astype(ml_dtypes.bfloat16))
    consts = np.zeros((128, C_COLS), dtype=np.float32)
    consts[:, C_W1A:C_W1A + 64] = w1[0:128]
    consts[:, C_W1B:C_W1B + 64] = w1[128:256]
    consts[0:64, C_W2:C_W2 + 8] = w2
    consts[0, C_B1:C_B1 + 64] = b1
    consts[0, C_B2:C_B2 + 8] = b2
    consts[0, C_ONES:C_ONES + 128] = 1.0
    mc = np.zeros((128, M_COLS), dtype=ml_dtypes.bfloat16)
    r = np.arange(128)
    for j in range(8):
        m = np.zeros((128, 128), dtype=np.float32)
        m[r, 16 * j + r // 8] = 1.0
        mc[:, M_MASK + j * 128:M_MASK + (j + 1) * 128] = m
    ks = np.zeros((8, 128), dtype=np.float32)
    ks[np.arange(8)[:, None] == np.arange(128)[None, :] % 8] = 1.0
    mc[0:8, M_KSEL:M_KSEL + 128] = ks
    return xh, bankh, consts, mc


def kernel(x, kernels, w1, b1, w2, b2):
    global _cached
    if _cached is None:
        _cached = _build()
    nc = _cached
    xh, bankh, consts, mc = _prep(np.asarray(x, dtype=np.float32),
                                  np.asarray(kernels, dtype=np.float32),
                                  np.asarray(w1, dtype=np.float32),
                                  np.asarray(b1, dtype=np.float32),
                                  np.asarray(w2, dtype=np.float32),
                                  np.asarray(b2, dtype=np.float32))
    in_maps = [{"x": xh[c * B:(c + 1) * B], "bank": bankh,
                "consts": consts, "mconsts": mc} for c in range(N_CORES)]
    res = run_bass_kernel_spmd(nc, in_maps, list(range(N_CORES)))
    out = np.concatenate(
        [res.results[c]["y"].reshape(B, C, H, W_IMG) for c in range(N_CORES)],
        axis=0)
    return out.astype(np.float32)
